# revision 17
# baseline (speedup 1.0000x reference)
"""Causal multi-head self-attention on 8 Trainium2 NeuronCores.

Sharding (2D): core c -> (batch b = c//2, head-group g = c%2 of 8 heads).
Each core computes the full attention for one batch and 8 heads (4
head-pair blocks of 128 qkv dims), then its partial output projection
out_p[b] = A_g @ Wo_g^T; the host sums the 2 partials per batch.
Per-core HBM traffic is ~12 MB (fp16) vs ~67 MB for pure head-parallel.

Per core:
  - x (one batch, [128, KT, S] transposed, fp16) stays resident in SBUF.
  - QT/KT = (x @ W^T)^T per head-pair block in transposed layout
    [128 dims, S tokens] (Wq pre-scaled by 1/sqrt(hd) on host).
  - V computed directly in natural [tokens, dims] layout by making the
    x-tile the stationary operand and streaming all 4 blocks of Wv
    (N=512) -- no PE transposes. A ones-column per head makes each
    attn@V matmul also emit the softmax denominator.
  - scores^T = K Q^T per (head-pair, q-chunk, k-tile); the two heads'
    K=64 matmuls auto-pack into row-tiles (0,0)/(64,0) and run
    concurrently. Causal via per-tile widths + one 128x128 staircase
    mask multiply on diagonal tiles. exp on ScalarE only; all PSUM
    evacuation on VectorE (fp16, 2x mode).
  - softmax denominators spread over 128 partitions via a DRAM bounce
    for the reciprocal, then broadcast back (stride-0 DRAM AP).
  - fused output projection per q-chunk once the last head-pair block's
    attention lands; out written as fp16.
"""

import numpy as np
from contextlib import ExitStack

import concourse.bass as bass
import concourse.mybir as mybir
import concourse.tile as tile
from concourse import bacc

F32 = mybir.dt.float32
F16 = mybir.dt.float16
EXP = mybir.ActivationFunctionType.Exp
MULT = mybir.AluOpType.mult
ADD = mybir.AluOpType.add


class Cfg:
    def __init__(self, B=4, S=2048, D=1024, QCH=512, mm_dt="f16"):
        self.B, self.S, self.D = B, S, D
        self.KT = D // 128          # contraction tiles for projections
        self.QCH = QCH              # query chunk for attention
        self.NQC = S // QCH         # q chunks per core
        self.HPB = 4                # head-pair blocks per core (8 heads)
        self.HD = 64
        self.NTT = S // 128         # token tiles per core
        self.mm_dt = mm_dt


def build_program(cfg: Cfg):
    nc = bacc.Bacc("TRN2", target_bir_lowering=False, debug=False)
    S, KT, QCH, NQC, HPB, NTT = (cfg.S, cfg.KT, cfg.QCH, cfg.NQC,
                                 cfg.HPB, cfg.NTT)
    D = cfg.D

    xT_d = nc.dram_tensor("xT", [128, KT, S], F16, kind="ExternalInput")
    wq_d = nc.dram_tensor("wq", [128, KT, HPB, 128], F16, kind="ExternalInput")
    wk_d = nc.dram_tensor("wk", [128, KT, HPB, 128], F16, kind="ExternalInput")
    wv_d = nc.dram_tensor("wv", [128, KT, HPB, 128], F16, kind="ExternalInput")
    wo_d = nc.dram_tensor("wo", [128, HPB, D], F16, kind="ExternalInput")
    mask_d = nc.dram_tensor("mask", [128, 128], F16, kind="ExternalInput")
    out_d = nc.dram_tensor("out_p", [S, D], F16, kind="ExternalOutput")
    out_r = out_d.rearrange("(n p) o -> p n o", p=128)   # [128, NTT, D]

    with tile.TileContext(nc) as tc, ExitStack() as ctx:
        persist = ctx.enter_context(tc.tile_pool(name="persist", bufs=1))

        x_sb = persist.tile([128, KT, S], F16, tag="x")
        qt_sb = persist.tile([128, HPB, S], F16, tag="qt")
        kt_sb = persist.tile([128, HPB, S], F16, tag="kt")
        a_sb = persist.tile([128, HPB, S], F16, tag="a")
        # V natural layout per (token-tile, head-pair block):
        #   cols 0:64 = even head dims, col 64 = 1.0,
        #   cols 65:129 = odd head dims, col 129 = 1.0
        v_sb = persist.tile([128, NTT, HPB, 130], F16, tag="v")
        wq_sb = persist.tile([128, KT, HPB, 128], F16, tag="wq")
        wk_sb = persist.tile([128, KT, HPB, 128], F16, tag="wk")
        wv_sb = persist.tile([128, KT, HPB, 128], F16, tag="wv")
        wo_sb = persist.tile([128, HPB, D], F16, tag="wo")
        mask_sb = persist.tile([128, 128], F16, tag="mask")
        ones128 = persist.tile([128, 1], F16, tag="ones128")
        nc.vector.memset(ones128[:], 1.0)

        nc.sync.dma_start(mask_sb[:], mask_d[:])
        for kt2 in range(0, KT, 2):
            nc.sync.dma_start(wv_sb[:, kt2:kt2 + 2], wv_d[:, kt2:kt2 + 2])
        # split x by token range so the first V-projection matmul can
        # start as soon as the first 256 tokens (+wv) have landed
        nc.sync.dma_start(x_sb[:, :, 0:512], xT_d[:, :, 0:512])
        for kt2 in range(0, KT, 4):
            nc.sync.dma_start(wq_sb[:, kt2:kt2 + 4], wq_d[:, kt2:kt2 + 4])
            nc.sync.dma_start(wk_sb[:, kt2:kt2 + 4], wk_d[:, kt2:kt2 + 4])
        for tch in range(512, S, 512):
            nc.sync.dma_start(x_sb[:, :, tch:tch + 512],
                              xT_d[:, :, tch:tch + 512])
        nc.sync.dma_start(wo_sb[:], wo_d[:])
        nc.vector.tensor_copy(
            v_sb[:, :, :, 64:65],
            ones128[:, None, None, :].to_broadcast((128, NTT, HPB, 1)))
        nc.vector.tensor_copy(
            v_sb[:, :, :, 129:130],
            ones128[:, None, None, :].to_broadcast((128, NTT, HPB, 1)))

        mm = ctx.enter_context(tc.tile_pool(name="mm", bufs=2, space="PSUM"))
        scp = ctx.enter_context(tc.tile_pool(name="scp", bufs=2, space="PSUM"))
        attp = ctx.enter_context(
            tc.tile_pool(name="attp", bufs=2, space="PSUM"))
        ptp = ctx.enter_context(tc.tile_pool(name="ptp", bufs=3))
        rcp = ctx.enter_context(tc.tile_pool(name="rcp", bufs=2))
        op = ctx.enter_context(tc.tile_pool(name="op", bufs=2))
        opp = ctx.enter_context(tc.tile_pool(name="opp", bufs=4))
        drp = ctx.enter_context(tc.tile_pool(name="drp", bufs=2, space="DRAM"))

        # ---------------- V projection, natural layout ------------------
        # out[t, (hp, m)] = sum_din x[t, din] * Wv[g*512 + hp*128 + m, din]
        for tt in range(NTT):
            ps = mm.tile([128, 512], F32, tag="mm")
            for kt in range(KT):
                nc.tensor.matmul(
                    ps[:],
                    x_sb[:, kt, tt * 128:(tt + 1) * 128],
                    wv_sb[:, kt, :, :],
                    start=(kt == 0), stop=(kt == KT - 1),
                )
            ps3 = ps.rearrange("p (h m) -> p h m", h=HPB)
            nc.scalar.copy(v_sb[:, tt, :, 0:64], ps3[:, :, 0:64])
            nc.scalar.copy(v_sb[:, tt, :, 65:129], ps3[:, :, 64:128])

        # ------------- per head-pair block: Q/K proj + attention --------
        def qk_proj(hp):
            for w_sb, dst in ((wq_sb, qt_sb), (wk_sb, kt_sb)):
                for tci in range(S // 512):
                    t0 = tci * 512
                    ps = mm.tile([128, 512], F32, tag="mm")
                    for kt in range(KT):
                        nc.tensor.matmul(
                            ps[:],
                            w_sb[:, kt, hp, :],
                            x_sb[:, kt, t0:t0 + 512],
                            start=(kt == 0), stop=(kt == KT - 1),
                        )
                    if hp == 0:
                        nc.scalar.copy(dst[:, hp, t0:t0 + 512], ps[:])
                    else:
                        nc.vector.tensor_copy(dst[:, hp, t0:t0 + 512], ps[:])

        def attention(hp, qc):
            q0 = qc * QCH
            n_kt = (q0 + QCH) // 128
            att0 = attp.tile([65, QCH], F32, tag="att")
            att1 = attp.tile([65, QCH], F32, tag="att")
            for kti in range(n_kt):
                k0 = kti * 128
                co = max(0, k0 - q0)
                sc = scp.tile([128, 2, QCH], F32, tag="sc")
                for h in (0, 1):
                    nc.tensor.matmul(
                        sc[:, h, co:QCH],
                        kt_sb[h * 64:(h + 1) * 64, hp, k0:k0 + 128],
                        qt_sb[h * 64:(h + 1) * 64, hp, q0 + co:q0 + QCH],
                        start=True, stop=True,
                    )
                pt = ptp.tile([128, 2, QCH], F16, tag="pt")
                nc.scalar.activation(pt[:, :, co:QCH], sc[:, :, co:QCH], EXP)
                if k0 >= q0:
                    st = pt[:, :, co:co + 128]
                    nc.vector.tensor_tensor(
                        st, st,
                        mask_sb[:, None, :].to_broadcast((128, 2, 128)),
                        MULT)
                nc.tensor.matmul(
                    att0[:, co:QCH],
                    v_sb[:, kti, hp, 0:65],
                    pt[:, 0, co:QCH],
                    start=(kti == 0), stop=(kti == n_kt - 1),
                )
                nc.tensor.matmul(
                    att1[:, co:QCH],
                    v_sb[:, kti, hp, 65:130],
                    pt[:, 1, co:QCH],
                    start=(kti == 0), stop=(kti == n_kt - 1),
                )
            # normalize: evacuate PSUM fast, reciprocal via DRAM bounce
            cols = slice(q0, q0 + QCH)
            au = rcp.tile([65, 2, QCH], F16, tag="au")
            au0 = au[:, 0, :]
            au1 = au[:, 1, :]
            nc.vector.tensor_copy(au0, att0[:])
            nc.vector.tensor_copy(au1, att1[:])
            NI = QCH // 128
            d_dn = drp.tile([2, QCH], F16, tag="ddn")
            nc.gpsimd.dma_start(d_dn[:, :], au[64:65, :, :])
            sp = rcp.tile([128, 2, NI], F16, tag="sp")
            nc.gpsimd.dma_start(
                sp[:], d_dn.rearrange("h (p i) -> p h i", p=128))
            rcs = rcp.tile([128, 2, NI], F32, tag="rcs")
            nc.vector.reciprocal(rcs[:], sp[:])
            d_rc = drp.tile([2, QCH], F32, tag="drc")
            nc.gpsimd.dma_start(
                d_rc.rearrange("h (p i) -> p h i", p=128), rcs[:])
            bc0 = rcp.tile([64, QCH], F32, tag="bc0")
            bc1 = rcp.tile([64, QCH], F32, tag="bc1")
            nc.gpsimd.dma_start(
                bc0[:], bass.AP(tensor=d_rc.tensor, offset=d_rc.offset,
                                ap=[[0, 64], [1, QCH]]))
            nc.gpsimd.dma_start(
                bc1[:], bass.AP(tensor=d_rc.tensor,
                                offset=d_rc.offset + QCH,
                                ap=[[0, 64], [1, QCH]]))
            nc.vector.tensor_tensor(
                a_sb[0:64, hp, cols], au0[0:64, :], bc0[:], MULT)
            a1_t = rcp.tile([64, QCH], F16, tag="a1")
            nc.vector.tensor_tensor(
                a1_t[:], au1[0:64, :], bc1[:], MULT)
            nc.sync.dma_start(a_sb[64:128, hp, cols], a1_t[:])

        def outproj_pre(qc, ti):
            # accumulate the head-pair blocks whose attention is already
            # done into an SBUF partial; the last block's contribution and
            # the add land in outproj_fin after that block's normalize
            tt = qc * QCH // 128 + ti
            ps0 = mm.tile([128, 512], F32, tag="mm")
            ps1 = mm.tile([128, 512], F32, tag="mm")
            for kb in range(HPB - 1):
                at = a_sb[:, kb, tt * 128:(tt + 1) * 128]
                nc.tensor.matmul(ps0[:], at, wo_sb[:, kb, 0:512],
                                 start=(kb == 0), stop=(kb == HPB - 2))
                nc.tensor.matmul(ps1[:], at, wo_sb[:, kb, 512:1024],
                                 start=(kb == 0), stop=(kb == HPB - 2))
            part = opp.tile([128, D], F32, tag="part")
            nc.vector.tensor_copy(part[:, 0:512], ps0[:])
            nc.vector.tensor_copy(part[:, 512:1024], ps1[:])
            return part

        def outproj_fin(qc, ti, part):
            tt = qc * QCH // 128 + ti
            kb = HPB - 1
            at = a_sb[:, kb, tt * 128:(tt + 1) * 128]
            ps0 = mm.tile([128, 512], F32, tag="mm")
            ps1 = mm.tile([128, 512], F32, tag="mm")
            nc.tensor.matmul(ps0[:], at, wo_sb[:, kb, 0:512],
                             start=True, stop=True)
            nc.tensor.matmul(ps1[:], at, wo_sb[:, kb, 512:1024],
                             start=True, stop=True)
            o_sb = op.tile([128, D], F16, tag="osb")
            nc.vector.tensor_tensor(o_sb[:, 0:512], ps0[:],
                                    part[:, 0:512], ADD)
            nc.vector.tensor_tensor(o_sb[:, 512:1024], ps1[:],
                                    part[:, 512:1024], ADD)
            nc.sync.dma_start(out_r[:, tt, :], o_sb[:])

        # per block: proj then attention; later blocks' projections fill
        # the PE during earlier blocks' ACT-bound attention. The last
        # block walks q-chunks descending so each chunk's normalize chain
        # + output projection hides under the next chunk's attention and
        # the kernel tail ends on the shortest chunk.
        for hp in range(HPB - 1):
            qk_proj(hp)
            for qc in range(NQC):
                attention(hp, qc)
        qk_proj(HPB - 1)
        for qc in range(NQC - 1, -1, -1):
            pre = [outproj_pre(qc, ti) for ti in range(QCH // 128)]
            attention(HPB - 1, qc)
            for ti in range(QCH // 128):
                outproj_fin(qc, ti, pre[ti])

    nc.compile()
    return nc


def prep_inputs(in_features, weight_q, weight_k, weight_v, weight_o, cfg: Cfg,
                n_cores=8):
    """Host-side shard/layout prep. Returns per-core input dicts."""
    S, D, KT, HPB = cfg.S, cfg.D, cfg.KT, cfg.HPB
    x = np.asarray(in_features, dtype=np.float32)       # [B, S, D]
    mask = np.triu(np.ones((128, 128), dtype=np.float16))
    wq = np.asarray(weight_q, dtype=np.float32) * (1.0 / np.sqrt(cfg.HD))
    wk = np.asarray(weight_k, dtype=np.float32)
    wv = np.asarray(weight_v, dtype=np.float32)
    wo = np.asarray(weight_o, dtype=np.float32)

    def wblock(w, g):
        # [128, KT, HPB, 128]: ws[p, kt, hp, m] = w[g*512+hp*128+m, kt*128+p]
        blk = w[g * 512:(g + 1) * 512, :]                 # [512, D]
        return np.ascontiguousarray(
            blk.T.reshape(KT, 128, HPB, 128).transpose(1, 0, 2, 3)
        ).astype(np.float16)

    def woblock(g):
        # [128, HPB, D]: wo_sb[p, kb, o] = Wo[o, g*512 + kb*128 + p]
        blk = wo[:, g * 512:(g + 1) * 512]                # [D, 512]
        return np.ascontiguousarray(
            blk.T.reshape(HPB, 128, D).transpose(1, 0, 2)).astype(np.float16)

    xTs = []
    for b in range(cfg.B):
        xb = x[b]                                         # [S, D]
        xTs.append(np.ascontiguousarray(
            xb.T.reshape(KT, 128, S).transpose(1, 0, 2)).astype(np.float16))

    in_maps = []
    for c in range(n_cores):
        b, g = c // 2, c % 2
        in_maps.append({
            "xT": xTs[b],
            "wq": wblock(wq, g),
            "wk": wblock(wk, g),
            "wv": wblock(wv, g),
            "wo": woblock(g),
            "mask": mask,
        })
    return in_maps


_CACHE = {}


def _get_program(cfg: Cfg):
    key = (cfg.B, cfg.S, cfg.D, cfg.QCH, cfg.mm_dt)
    if key not in _CACHE:
        _CACHE[key] = build_program(cfg)
    return _CACHE[key]


def run(inputs, cfg: Cfg, trace=False, trace_kwargs=None):
    import time
    from concourse.bass_utils import run_bass_kernel_spmd
    nc = _get_program(cfg)
    in_maps = prep_inputs(**inputs, cfg=cfg)
    last = None
    for attempt in range(3):
        try:
            res = run_bass_kernel_spmd(
                nc, in_maps, core_ids=list(range(8)), trace=trace,
                **(trace_kwargs or {}))
            break
        except Exception as e:  # transient NRT device wedges happen
            last = e
            time.sleep(10)
    else:
        raise last
    parts = [r["out_p"].astype(np.float32) for r in res.results]
    out = np.stack([parts[2 * b] + parts[2 * b + 1] for b in range(cfg.B)], 0)
    return out.astype(np.float32), res


def kernel(in_features, weight_q, weight_k, weight_v, weight_o):
    cfg = Cfg()
    out, _ = run(dict(in_features=in_features, weight_q=weight_q,
                      weight_k=weight_k, weight_v=weight_v,
                      weight_o=weight_o), cfg)
    return out


# revision 20
# speedup vs baseline: 1.0149x; 1.0149x over previous
"""Causal multi-head self-attention on 8 Trainium2 NeuronCores.

Sharding (2D): core c -> (batch b = c//2, head-group g = c%2 of 8 heads).
Each core computes the full attention for one batch and 8 heads (4
head-pair blocks of 128 qkv dims), then its partial output projection
out_p[b] = A_g @ Wo_g^T; the host sums the 2 partials per batch.
Per-core HBM traffic is ~12 MB (fp16) vs ~67 MB for pure head-parallel.

Per core:
  - x (one batch, [128, KT, S] transposed, fp16) stays resident in SBUF.
  - QT/KT = (x @ W^T)^T per head-pair block in transposed layout
    [128 dims, S tokens] (Wq pre-scaled by 1/sqrt(hd) on host).
  - V computed directly in natural [tokens, dims] layout by making the
    x-tile the stationary operand and streaming all 4 blocks of Wv
    (N=512) -- no PE transposes. A ones-column per head makes each
    attn@V matmul also emit the softmax denominator.
  - scores^T = K Q^T per (head-pair, q-chunk, k-tile); the two heads'
    K=64 matmuls auto-pack into row-tiles (0,0)/(64,0) and run
    concurrently. Causal via per-tile widths + one 128x128 staircase
    mask multiply on diagonal tiles. exp on ScalarE only; all PSUM
    evacuation on VectorE (fp16, 2x mode).
  - softmax denominators spread over 128 partitions via a DRAM bounce
    for the reciprocal, then broadcast back (stride-0 DRAM AP).
  - fused output projection per q-chunk once the last head-pair block's
    attention lands; out written as fp16.
"""

import numpy as np
from contextlib import ExitStack

import concourse.bass as bass
import concourse.mybir as mybir
import concourse.tile as tile
from concourse import bacc

F32 = mybir.dt.float32
F16 = mybir.dt.float16
EXP = mybir.ActivationFunctionType.Exp
MULT = mybir.AluOpType.mult
ADD = mybir.AluOpType.add


class Cfg:
    def __init__(self, B=4, S=2048, D=1024, QCH=512, mm_dt="f16"):
        self.B, self.S, self.D = B, S, D
        self.KT = D // 128          # contraction tiles for projections
        self.QCH = QCH              # query chunk for attention
        self.NQC = S // QCH         # q chunks per core
        self.HPB = 4                # head-pair blocks per core (8 heads)
        self.HD = 64
        self.NTT = S // 128         # token tiles per core
        self.mm_dt = mm_dt


def build_program(cfg: Cfg):
    nc = bacc.Bacc("TRN2", target_bir_lowering=False, debug=False)
    S, KT, QCH, NQC, HPB, NTT = (cfg.S, cfg.KT, cfg.QCH, cfg.NQC,
                                 cfg.HPB, cfg.NTT)
    D = cfg.D

    xT_d = nc.dram_tensor("xT", [128, KT, S], F16, kind="ExternalInput")
    wq_d = nc.dram_tensor("wq", [128, KT, HPB, 128], F16, kind="ExternalInput")
    wk_d = nc.dram_tensor("wk", [128, KT, HPB, 128], F16, kind="ExternalInput")
    wv_d = nc.dram_tensor("wv", [128, KT, HPB, 128], F16, kind="ExternalInput")
    wo_d = nc.dram_tensor("wo", [128, HPB, D], F16, kind="ExternalInput")
    mask_d = nc.dram_tensor("mask", [128, 128], F16, kind="ExternalInput")
    out_d = nc.dram_tensor("out_p", [S, D], F16, kind="ExternalOutput")
    out_r = out_d.rearrange("(n p) o -> p n o", p=128)   # [128, NTT, D]

    with tile.TileContext(nc) as tc, ExitStack() as ctx:
        persist = ctx.enter_context(tc.tile_pool(name="persist", bufs=1))

        x_sb = persist.tile([128, KT, S], F16, tag="x")
        qt_sb = persist.tile([128, HPB, S], F16, tag="qt")
        kt_sb = persist.tile([128, HPB, S], F16, tag="kt")
        a_sb = persist.tile([128, HPB, S], F16, tag="a")
        # V natural layout per (token-tile, head-pair block):
        #   cols 0:64 = even head dims, col 64 = 1.0,
        #   cols 65:129 = odd head dims, col 129 = 1.0
        v_sb = persist.tile([128, NTT, HPB, 130], F16, tag="v")
        wq_sb = persist.tile([128, KT, HPB, 128], F16, tag="wq")
        wk_sb = persist.tile([128, KT, HPB, 128], F16, tag="wk")
        wv_sb = persist.tile([128, KT, HPB, 128], F16, tag="wv")
        wo_sb = persist.tile([128, HPB, D], F16, tag="wo")
        mask_sb = persist.tile([128, 128], F16, tag="mask")
        ones128 = persist.tile([128, 1], F16, tag="ones128")
        nc.vector.memset(ones128[:], 1.0)

        nc.sync.dma_start(mask_sb[:], mask_d[:])
        for kt2 in range(0, KT, 2):
            nc.sync.dma_start(wv_sb[:, kt2:kt2 + 2], wv_d[:, kt2:kt2 + 2])
        # split x by token range so the first V-projection matmul can
        # start as soon as the first 256 tokens (+wv) have landed
        nc.sync.dma_start(x_sb[:, :, 0:512], xT_d[:, :, 0:512])
        for kt2 in range(0, KT, 4):
            nc.sync.dma_start(wq_sb[:, kt2:kt2 + 4], wq_d[:, kt2:kt2 + 4])
            nc.sync.dma_start(wk_sb[:, kt2:kt2 + 4], wk_d[:, kt2:kt2 + 4])
        for tch in range(512, S, 512):
            nc.sync.dma_start(x_sb[:, :, tch:tch + 512],
                              xT_d[:, :, tch:tch + 512])
        nc.sync.dma_start(wo_sb[:], wo_d[:])
        nc.vector.tensor_copy(
            v_sb[:, :, :, 64:65],
            ones128[:, None, None, :].to_broadcast((128, NTT, HPB, 1)))
        nc.vector.tensor_copy(
            v_sb[:, :, :, 129:130],
            ones128[:, None, None, :].to_broadcast((128, NTT, HPB, 1)))

        mm = ctx.enter_context(tc.tile_pool(name="mm", bufs=2, space="PSUM"))
        scp = ctx.enter_context(tc.tile_pool(name="scp", bufs=2, space="PSUM"))
        attp = ctx.enter_context(
            tc.tile_pool(name="attp", bufs=2, space="PSUM"))
        ptp = ctx.enter_context(tc.tile_pool(name="ptp", bufs=3))
        rcp = ctx.enter_context(tc.tile_pool(name="rcp", bufs=2))
        op = ctx.enter_context(tc.tile_pool(name="op", bufs=2))
        opp = ctx.enter_context(tc.tile_pool(name="opp", bufs=4))
        drp = ctx.enter_context(tc.tile_pool(name="drp", bufs=2, space="DRAM"))

        # ---------------- V projection, natural layout ------------------
        # out[t, (hp, m)] = sum_din x[t, din] * Wv[g*512 + hp*128 + m, din]
        for tt in range(NTT):
            ps = mm.tile([128, 512], F32, tag="mm")
            for kt in range(KT):
                nc.tensor.matmul(
                    ps[:],
                    x_sb[:, kt, tt * 128:(tt + 1) * 128],
                    wv_sb[:, kt, :, :],
                    start=(kt == 0), stop=(kt == KT - 1),
                )
            ps3 = ps.rearrange("p (h m) -> p h m", h=HPB)
            nc.scalar.copy(v_sb[:, tt, :, 0:64], ps3[:, :, 0:64])
            nc.scalar.copy(v_sb[:, tt, :, 65:129], ps3[:, :, 64:128])

        # ------------- per head-pair block: Q/K proj + attention --------
        def qk_proj(hp):
            for w_sb, dst in ((wq_sb, qt_sb), (wk_sb, kt_sb)):
                for tci in range(S // 512):
                    t0 = tci * 512
                    ps = mm.tile([128, 512], F32, tag="mm")
                    for kt in range(KT):
                        nc.tensor.matmul(
                            ps[:],
                            w_sb[:, kt, hp, :],
                            x_sb[:, kt, t0:t0 + 512],
                            start=(kt == 0), stop=(kt == KT - 1),
                        )
                    if hp == 0:
                        nc.scalar.copy(dst[:, hp, t0:t0 + 512], ps[:])
                    else:
                        nc.vector.tensor_copy(dst[:, hp, t0:t0 + 512], ps[:])

        def attention(hp, qc):
            q0 = qc * QCH
            n_kt = (q0 + QCH) // 128
            att0 = attp.tile([65, QCH], F32, tag="att")
            att1 = attp.tile([65, QCH], F32, tag="att")
            for kti in range(n_kt):
                k0 = kti * 128
                co = max(0, k0 - q0)
                sc = scp.tile([128, 2, QCH], F32, tag="sc")
                for h in (0, 1):
                    nc.tensor.matmul(
                        sc[:, h, co:QCH],
                        kt_sb[h * 64:(h + 1) * 64, hp, k0:k0 + 128],
                        qt_sb[h * 64:(h + 1) * 64, hp, q0 + co:q0 + QCH],
                        start=True, stop=True,
                    )
                pt = ptp.tile([128, 2, QCH], F16, tag="pt")
                nc.scalar.activation(pt[:, :, co:QCH], sc[:, :, co:QCH], EXP)
                if k0 >= q0:
                    st = pt[:, :, co:co + 128]
                    nc.vector.tensor_tensor(
                        st, st,
                        mask_sb[:, None, :].to_broadcast((128, 2, 128)),
                        MULT)
                nc.tensor.matmul(
                    att0[:, co:QCH],
                    v_sb[:, kti, hp, 0:65],
                    pt[:, 0, co:QCH],
                    start=(kti == 0), stop=(kti == n_kt - 1),
                )
                nc.tensor.matmul(
                    att1[:, co:QCH],
                    v_sb[:, kti, hp, 65:130],
                    pt[:, 1, co:QCH],
                    start=(kti == 0), stop=(kti == n_kt - 1),
                )
            # normalize: evacuate PSUM fast, reciprocal via DRAM bounce
            cols = slice(q0, q0 + QCH)
            au = rcp.tile([65, 2, QCH], F16, tag="au")
            au0 = au[:, 0, :]
            au1 = au[:, 1, :]
            nc.vector.tensor_copy(au0, att0[:])
            nc.vector.tensor_copy(au1, att1[:])
            NI = QCH // 128
            d_dn = drp.tile([2, QCH], F16, tag="ddn")
            nc.gpsimd.dma_start(d_dn[:, :], au[64:65, :, :])
            sp = rcp.tile([128, 2, NI], F16, tag="sp")
            nc.gpsimd.dma_start(
                sp[:], d_dn.rearrange("h (p i) -> p h i", p=128))
            rcs = rcp.tile([128, 2, NI], F32, tag="rcs")
            nc.vector.reciprocal(rcs[:], sp[:])
            d_rc = drp.tile([2, QCH], F32, tag="drc")
            nc.gpsimd.dma_start(
                d_rc.rearrange("h (p i) -> p h i", p=128), rcs[:])
            bc0 = rcp.tile([64, QCH], F32, tag="bc0")
            bc1 = rcp.tile([64, QCH], F32, tag="bc1")
            nc.gpsimd.dma_start(
                bc0[:], bass.AP(tensor=d_rc.tensor, offset=d_rc.offset,
                                ap=[[0, 64], [1, QCH]]))
            nc.gpsimd.dma_start(
                bc1[:], bass.AP(tensor=d_rc.tensor,
                                offset=d_rc.offset + QCH,
                                ap=[[0, 64], [1, QCH]]))
            nc.vector.tensor_tensor(
                a_sb[0:64, hp, cols], au0[0:64, :], bc0[:], MULT)
            a1_t = rcp.tile([64, QCH], F16, tag="a1")
            nc.vector.tensor_tensor(
                a1_t[:], au1[0:64, :], bc1[:], MULT)
            nc.sync.dma_start(a_sb[64:128, hp, cols], a1_t[:])

        def outproj_pre(qc, ti):
            # accumulate the head-pair blocks whose attention is already
            # done into an SBUF partial; the last block's contribution and
            # the add land in outproj_fin after that block's normalize
            tt = qc * QCH // 128 + ti
            ps0 = mm.tile([128, 512], F32, tag="mm")
            ps1 = mm.tile([128, 512], F32, tag="mm")
            for kb in range(HPB - 1):
                at = a_sb[:, kb, tt * 128:(tt + 1) * 128]
                nc.tensor.matmul(ps0[:], at, wo_sb[:, kb, 0:512],
                                 start=(kb == 0), stop=(kb == HPB - 2))
                nc.tensor.matmul(ps1[:], at, wo_sb[:, kb, 512:1024],
                                 start=(kb == 0), stop=(kb == HPB - 2))
            part = opp.tile([128, D], F32, tag="part")
            nc.vector.tensor_copy(part[:, 0:512], ps0[:])
            nc.vector.tensor_copy(part[:, 512:1024], ps1[:])
            return part

        def outproj_fin(qc, ti, part):
            tt = qc * QCH // 128 + ti
            kb = HPB - 1
            at = a_sb[:, kb, tt * 128:(tt + 1) * 128]
            ps0 = mm.tile([128, 512], F32, tag="mm")
            ps1 = mm.tile([128, 512], F32, tag="mm")
            nc.tensor.matmul(ps0[:], at, wo_sb[:, kb, 0:512],
                             start=True, stop=True)
            nc.tensor.matmul(ps1[:], at, wo_sb[:, kb, 512:1024],
                             start=True, stop=True)
            o_sb = op.tile([128, D], F16, tag="osb")
            nc.vector.tensor_tensor(o_sb[:, 0:512], ps0[:],
                                    part[:, 0:512], ADD)
            nc.vector.tensor_tensor(o_sb[:, 512:1024], ps1[:],
                                    part[:, 512:1024], ADD)
            nc.sync.dma_start(out_r[:, tt, :], o_sb[:])

        # per block: proj then attention; later blocks' projections fill
        # the PE during earlier blocks' ACT-bound attention. The last
        # block walks q-chunks descending so each chunk's normalize chain
        # + output projection hides under the next chunk's attention and
        # the kernel tail ends on the shortest chunk.
        for hp in range(HPB - 1):
            qk_proj(hp)
            for qc in range(NQC):
                attention(hp, qc)
        qk_proj(HPB - 1)
        for qc in range(NQC - 1, -1, -1):
            attention(HPB - 1, qc)
            # pre MMs land on the PE queue right after this chunk's attnV
            # matmuls, so they execute during the normalize chain's DMA
            # round-trips; only the tiny fin MMs wait on the chain
            pre = [outproj_pre(qc, ti) for ti in range(QCH // 128)]
            for ti in range(QCH // 128):
                outproj_fin(qc, ti, pre[ti])

    nc.compile()
    return nc


def prep_inputs(in_features, weight_q, weight_k, weight_v, weight_o, cfg: Cfg,
                n_cores=8):
    """Host-side shard/layout prep. Returns per-core input dicts."""
    S, D, KT, HPB = cfg.S, cfg.D, cfg.KT, cfg.HPB
    x = np.asarray(in_features, dtype=np.float32)       # [B, S, D]
    mask = np.triu(np.ones((128, 128), dtype=np.float16))
    wq = np.asarray(weight_q, dtype=np.float32) * (1.0 / np.sqrt(cfg.HD))
    wk = np.asarray(weight_k, dtype=np.float32)
    wv = np.asarray(weight_v, dtype=np.float32)
    wo = np.asarray(weight_o, dtype=np.float32)

    def wblock(w, g):
        # [128, KT, HPB, 128]: ws[p, kt, hp, m] = w[g*512+hp*128+m, kt*128+p]
        blk = w[g * 512:(g + 1) * 512, :]                 # [512, D]
        return np.ascontiguousarray(
            blk.T.reshape(KT, 128, HPB, 128).transpose(1, 0, 2, 3)
        ).astype(np.float16)

    def woblock(g):
        # [128, HPB, D]: wo_sb[p, kb, o] = Wo[o, g*512 + kb*128 + p]
        blk = wo[:, g * 512:(g + 1) * 512]                # [D, 512]
        return np.ascontiguousarray(
            blk.T.reshape(HPB, 128, D).transpose(1, 0, 2)).astype(np.float16)

    xTs = []
    for b in range(cfg.B):
        xb = x[b]                                         # [S, D]
        xTs.append(np.ascontiguousarray(
            xb.T.reshape(KT, 128, S).transpose(1, 0, 2)).astype(np.float16))

    in_maps = []
    for c in range(n_cores):
        b, g = c // 2, c % 2
        in_maps.append({
            "xT": xTs[b],
            "wq": wblock(wq, g),
            "wk": wblock(wk, g),
            "wv": wblock(wv, g),
            "wo": woblock(g),
            "mask": mask,
        })
    return in_maps


_CACHE = {}


def _get_program(cfg: Cfg):
    key = (cfg.B, cfg.S, cfg.D, cfg.QCH, cfg.mm_dt)
    if key not in _CACHE:
        _CACHE[key] = build_program(cfg)
    return _CACHE[key]


def run(inputs, cfg: Cfg, trace=False, trace_kwargs=None):
    import time
    from concourse.bass_utils import run_bass_kernel_spmd
    nc = _get_program(cfg)
    in_maps = prep_inputs(**inputs, cfg=cfg)
    last = None
    for attempt in range(3):
        try:
            res = run_bass_kernel_spmd(
                nc, in_maps, core_ids=list(range(8)), trace=trace,
                **(trace_kwargs or {}))
            break
        except Exception as e:  # transient NRT device wedges happen
            last = e
            time.sleep(10)
    else:
        raise last
    parts = [r["out_p"].astype(np.float32) for r in res.results]
    out = np.stack([parts[2 * b] + parts[2 * b + 1] for b in range(cfg.B)], 0)
    return out.astype(np.float32), res


def kernel(in_features, weight_q, weight_k, weight_v, weight_o):
    cfg = Cfg()
    out, _ = run(dict(in_features=in_features, weight_q=weight_q,
                      weight_k=weight_k, weight_v=weight_v,
                      weight_o=weight_o), cfg)
    return out
